# revision 26
# baseline (speedup 1.0000x reference)
"""Single-head causal attention on 8 Trainium2 NeuronCores.

Problem: x:[8,2048,1024], Wq/Wk/Wv:[64,1024], bq/bk/bv:[64]
  q,k,v = x@W*.T + b*;  out = softmax(causal(q@k.T)/sqrt(64)) @ v

Sharding: batch dim (8) across the 8 cores — fully data-parallel, no
collectives. Each core computes one batch's attention head.

Per-core device kernel (all matmuls fp32r = tf32, full-rate at N=512):
  - host supplies x transposed (xT [E,S]) so E (the contraction dim of the
    projections) lands on SBUF partitions; x streams over two DMA queues
    (sync + gpsimd) in [128,512] tiles.
  - projections: psum_qk[128,512] accumulates [Wq|Wk].T packed (M=128),
    psum_v[64,512] accumulates Wv.T, over 8 e-tiles of 128.
  - q is pre-scaled by 1/sqrt(64) (scale folded into the PSUM->SBUF copy,
    bias folded there too), so scores come out pre-scaled.
  - k must sit at partitions 0:64 for the scores matmul but lands at 64:128
    of the packed projection; it is re-based with a constant permutation
    matmul (cheaper than an SBUF->SBUF DMA competing with the x stream).
  - scores kept transposed: sT[k,q] = kT.T @ qT per (k-tile 128, q-chunk 512).
    Causally-dead tiles are skipped outright; diagonal tiles are masked by
    multiplying exp(s) with a 0/1 ramp mask.
  - softmax without max-subtraction (scores/8 ~ N(0,1); max < ~6, exp safe
    in fp32) -> denominator = column sum of exp(sT), obtained for free as
    row 64 of the attention*V matmul by appending a ones-row to V.
  - V needs k on partitions for the AV matmul: vT tiles are transposed
    128-column-wise on the PE (identity matmul).
  - normalize on device: reciprocal of the denominator row, broadcast
    across the 64 head partitions with a K=1 matmul, multiply.
  - output written h-major ([64, 2048]); host transposes back.
  - emission is interleaved with generators: proj(c) and attn(c-1) alternate
    so the in-order engine queues see attention work during DMA waits, and
    scores run LOOKAHEAD k-tiles ahead of the AV consumer.
"""

import numpy as np

import concourse.bacc as bacc
import concourse.mybir as mybir
import concourse.tile as tile
from concourse import bass2jax

B, S, E, H = 8, 2048, 1024, 64
NCORES = 8
PB = 128  # partition block / k-tile size
QB = 512  # q-chunk (matmul moving free dim)
ET = E // PB  # e-tiles per contraction
QC = S // QB  # q-chunks
KT = S // PB  # k-tiles
DIAG = QB // PB  # diagonal k-tiles per q-chunk

# packed constants layout: columns of the [128, NCONST] "consts" input
C_SB = 0  # [*, 0:2]   scale/bias (rows 0:128)
C_BV = 2  # [*, 2:3]   v bias (rows 0:64)
C_ID = 3  # [*, 3:67]  identity 64x64 (rows 0:64)
C_ONES = 67  # [*, 67:131] ones
C_PERM = 131  # [*, 131:195] row-rebase permutation (p, h) = 1 iff p == h+64
C_MASK = 195  # [*, 195:1091] causal ramp mask: (p, j) = 1 iff j >= p + 384
NCONST = C_MASK + QB + 384

F32 = mybir.dt.float32
F32R = mybir.dt.float32r
AF = mybir.ActivationFunctionType
MUL = mybir.AluOpType.mult

_CACHE: dict = {}

# schedule/buffering knobs (sweepable)
CFG = {
    "lookahead": 5,
    "xbufs": 12,
    "wtbufs": 8,
    "psbufs": 4,
    "dma2": "gpsimd",  # second x-stream queue
    "wqkv_q": "gpsimd",
    "diag_first": False,
    "attn_first": False,
}


def _interleave(*gens):
    """Drive generators round-robin; the first (proj) gets two steps per turn."""
    alive = list(gens)
    steps = {id(g): (2 if i == 0 and len(gens) > 1 else 1) for i, g in enumerate(gens)}
    while alive:
        for g in list(alive):
            for _ in range(steps[id(g)]):
                try:
                    next(g)
                except StopIteration:
                    alive.remove(g)
                    break


def _build_nc():
    nc = bacc.Bacc("TRN2", target_bir_lowering=False, debug=False)
    xT = nc.dram_tensor("xT", [E, S], F32R, kind="ExternalInput").ap()
    wqkv = nc.dram_tensor("wqkv", [E, 3 * H], F32R, kind="ExternalInput").ap()
    consts = nc.dram_tensor("consts", [PB, NCONST], F32R, kind="ExternalInput").ap()
    out = nc.dram_tensor("out", [H, S], F32, kind="ExternalOutput").ap()

    with tile.TileContext(nc) as tc:
        with (
            tc.tile_pool(name="const", bufs=1) as constp,
            tc.tile_pool(name="xs", bufs=CFG["xbufs"]) as xpool,
            tc.tile_pool(name="qkv", bufs=1) as qkvp,
            tc.tile_pool(name="wt", bufs=CFG["wtbufs"]) as wtp,
            tc.tile_pool(name="fin", bufs=2) as finp,
            tc.tile_pool(name="pqk", bufs=1, space="PSUM") as pqk,
            tc.tile_pool(name="pv", bufs=1, space="PSUM") as pvp,
            tc.tile_pool(name="ps", bufs=CFG["psbufs"], space="PSUM") as psp,
            tc.tile_pool(name="pav", bufs=2, space="PSUM") as pavp,
        ):
            # constants ride the gpsimd queue so the sync queue starts on x;
            # cs is not needed until the first PSUM->SBUF copy.
            wqkv_sb = constp.tile([PB, ET, 3 * H], F32R)
            _wq = getattr(nc, CFG["wqkv_q"])
            _wq.dma_start(wqkv_sb[:], wqkv.rearrange("(t p) m -> p t m", p=PB))
            cs = constp.tile([PB, NCONST], F32R)
            nc.gpsimd.dma_start(cs[:], consts[:])

            scale_ap = cs[:, C_SB : C_SB + 1].bitcast(F32)
            bias_ap = cs[:, C_SB + 1 : C_SB + 2].bitcast(F32)
            bv_ap = cs[0:H, C_BV : C_BV + 1].bitcast(F32)
            id_ap = cs[0:H, C_ID : C_ID + H].bitcast(F32)
            ones_ap = cs[:, C_ONES : C_ONES + H]
            perm_ap = cs[:, C_PERM : C_PERM + H]

            qkT = qkvp.tile([PB, S], F32R)  # rows 0:64 = q/8, 64:128 = k
            kT = qkvp.tile([H, S], F32R)  # k re-based to partitions 0:64
            vT = qkvp.tile([H, S], F32)  # v h-major (bias applied)
            vsb = qkvp.tile([PB, KT, H + 1], F32R)  # v k-major + ones col
            for m in range(KT):
                nc.vector.tensor_copy(vsb[:, m, H : H + 1], ones_ap[:, 0:1])

            def proj(c):
                qs = slice(c * QB, (c + 1) * QB)
                p_qk = pqk.tile([PB, QB], F32, tag="pqk")
                p_v = pvp.tile([H, QB], F32, tag="pv")
                for e in range(ET):
                    xt = xpool.tile([PB, QB], F32R, tag="xt")
                    dma_eng = nc.sync if e % 2 == 0 else getattr(nc, CFG["dma2"])
                    dma_eng.dma_start(xt[:], xT[e * PB : (e + 1) * PB, qs])
                    nc.tensor.matmul(
                        p_qk[:],
                        wqkv_sb[:, e, 0 : 2 * H],
                        xt[:],
                        start=(e == 0),
                        stop=(e == ET - 1),
                    )
                    nc.tensor.matmul(
                        p_v[:],
                        wqkv_sb[:, e, 2 * H : 3 * H],
                        xt[:],
                        start=(e == 0),
                        stop=(e == ET - 1),
                    )
                    yield
                nc.scalar.activation(
                    qkT[:, qs], p_qk[:], AF.Identity, bias=bias_ap, scale=scale_ap
                )
                nc.scalar.activation(vT[:, qs], p_v[:], AF.Identity, bias=bv_ap)
                yield
                # re-base k rows 64:128 -> 0:64 via permutation matmul
                p_k = pqk.tile([PB, QB], F32, tag="pqk")
                nc.tensor.matmul(p_k[0:H, :], perm_ap, qkT[:, qs], start=True, stop=True)
                nc.vector.tensor_copy(kT[:, qs], p_k[0:H, :])
                yield
                for t in range(DIAG):
                    m = DIAG * c + t
                    p_vt = pvp.tile([PB, H], F32, tag="pv")
                    nc.tensor.transpose(p_vt[:], vT[:, m * PB : (m + 1) * PB], id_ap)
                    nc.vector.tensor_copy(vsb[:, m, 0:H], p_vt[:])
                    if t % 2 == 1:
                        yield

            def attn(c):
                qs = slice(c * QB, (c + 1) * QB)
                nkt = DIAG * c + DIAG
                p_av = pavp.tile([H + 1, QB], F32, tag="pav")

                def weights_tile(m):
                    # scores -> exp -> (diagonal) causal mask
                    p_s = psp.tile([PB, QB], F32, tag="ps")
                    nc.tensor.matmul(
                        p_s[:],
                        kT[:, m * PB : (m + 1) * PB],
                        qkT[0:H, qs],
                        start=True,
                        stop=True,
                    )
                    w = wtp.tile([PB, QB], F32R, tag="w")
                    nc.scalar.activation(w[:], p_s[:], AF.Exp)
                    i = m - DIAG * c
                    if i >= 0:
                        nc.vector.tensor_tensor(
                            w[:],
                            w[:],
                            cs[:, C_MASK + 384 - PB * i : C_MASK + 384 - PB * i + QB],
                            MUL,
                        )
                    return w

                L = CFG["lookahead"]
                if CFG["diag_first"]:
                    order = list(range(DIAG * c, nkt)) + list(range(0, DIAG * c))
                else:
                    order = list(range(nkt))
                ws = {m: weights_tile(m) for m in order[: min(L, nkt)]}
                yield
                for idx, m in enumerate(order):
                    if idx + L < nkt:
                        ws[order[idx + L]] = weights_tile(order[idx + L])
                    nc.tensor.matmul(
                        p_av[:],
                        vsb[:, m, :],
                        ws.pop(m),
                        start=(idx == 0),
                        stop=(idx == nkt - 1),
                    )
                    yield
                # normalize: out[h, q] = av[h, q] / av[64, q]
                dn = wtp.tile([PB, QB], F32R, tag="w")
                with nc.allow_low_precision("fp32r denominators feed an fp32r matmul"):
                    nc.vector.reciprocal(dn[H : H + 1, :], p_av[H : H + 1, :])
                p_rep = pavp.tile([H + 1, QB], F32, tag="pav")
                nc.tensor.matmul(
                    p_rep[0:H, :],
                    ones_ap[H : H + 1, :],
                    dn[H : H + 1, :],
                    start=True,
                    stop=True,
                )
                yield
                rep = finp.tile([H, QB], F32, tag="rep")
                nc.vector.tensor_copy(rep[:], p_rep[0:H, :])
                osb = finp.tile([H, QB], F32, tag="osb")
                nc.vector.tensor_tensor(osb[:], p_av[0:H, :], rep[:], MUL)
                nc.sync.dma_start(out[:, qs], osb[:])
                yield

            # interleaved emission: proj(c) alternates with attn(c-1) so the
            # in-order engine queues see attention work during DMA waits.
            _interleave(proj(0))
            for c in range(1, QC):
                if CFG["attn_first"]:
                    _interleave(attn(c - 1), proj(c))
                else:
                    _interleave(proj(c), attn(c - 1))
            _interleave(attn(QC - 1))

    nc.compile()
    return nc


def _host_inputs(x, Wq, bq, Wk, bk, Wv, bv):
    x = np.asarray(x, np.float32)
    Wq, bq = np.asarray(Wq, np.float32), np.asarray(bq, np.float32)
    Wk, bk = np.asarray(Wk, np.float32), np.asarray(bk, np.float32)
    Wv, bv = np.asarray(Wv, np.float32), np.asarray(bv, np.float32)

    sc = np.float32(1.0 / np.sqrt(H))
    wqkv = np.ascontiguousarray(np.concatenate([Wq.T, Wk.T, Wv.T], axis=1))  # [E, 3H]

    cs = np.zeros((PB, NCONST), np.float32)
    cs[:, C_SB] = np.concatenate([np.full(H, sc, np.float32), np.ones(H, np.float32)])
    cs[:, C_SB + 1] = np.concatenate([bq * sc, bk])
    cs[:H, C_BV] = bv
    cs[:H, C_ID : C_ID + H] = np.eye(H, dtype=np.float32)
    cs[:, C_ONES : C_ONES + H] = 1.0
    cs[H:PB, C_PERM : C_PERM + H] = np.eye(H, dtype=np.float32)
    j = np.arange(QB + 384, dtype=np.int64)[None, :]
    k = np.arange(PB, dtype=np.int64)[:, None]
    cs[:, C_MASK:] = (j >= k + 384).astype(np.float32)

    shared = {"wqkv": wqkv, "consts": cs}
    in_maps = []
    for b in range(B):
        m = dict(shared)
        m["xT"] = np.ascontiguousarray(x[b].T)
        in_maps.append(m)
    return in_maps


def get_nc():
    if "nc" not in _CACHE:
        _CACHE["nc"] = _build_nc()
    return _CACHE["nc"]


def kernel(x, Wq, bq, Wk, bk, Wv, bv):
    nc = get_nc()
    in_maps = _host_inputs(x, Wq, bq, Wk, bk, Wv, bv)
    results = bass2jax.run_bass_via_pjrt(nc, in_maps, n_cores=NCORES)
    out = np.empty((B, S, H), np.float32)
    for b in range(B):
        out[b] = results[b]["out"].T
    return out


# revision 49
# speedup vs baseline: 1.0547x; 1.0547x over previous
"""Single-head causal attention on 8 Trainium2 NeuronCores.

Problem: x:[8,2048,1024], Wq/Wk/Wv:[64,1024], bq/bk/bv:[64]
  q,k,v = x@W*.T + b*;  out = softmax(causal(q@k.T)/sqrt(64)) @ v

Sharding: batch dim (8) across the 8 cores — fully data-parallel, no
collectives. Each core computes one batch's attention head.

Per-core device kernel (all matmuls fp32r = tf32, full-rate at N=512):
  - host supplies x transposed (xT [E,S]) so E (the contraction dim of the
    projections) lands on SBUF partitions; x streams over two DMA queues
    (sync + gpsimd) in [128,512] tiles.
  - projections: psum_qk[128,512] accumulates [Wq|Wk].T packed (M=128),
    psum_v[64,512] accumulates Wv.T, over 8 e-tiles of 128.
  - q is pre-scaled by 1/sqrt(64) (scale folded into the PSUM->SBUF copy,
    bias folded there too), so scores come out pre-scaled.
  - k must sit at partitions 0:64 for the scores matmul but lands at 64:128
    of the packed projection; it is re-based with a constant permutation
    matmul (cheaper than an SBUF->SBUF DMA competing with the x stream).
  - scores kept transposed: sT[k,q] = kT.T @ qT per (k-tile 128, q-chunk 512).
    Causally-dead tiles are skipped outright; diagonal tiles are masked by
    multiplying exp(s) with a 0/1 ramp mask.
  - softmax without max-subtraction (scores/8 ~ N(0,1); max < ~6, exp safe
    in fp32) -> denominator = column sum of exp(sT), obtained for free as
    row 64 of the attention*V matmul by appending a ones-row to V.
  - V needs k on partitions for the AV matmul: vT tiles are transposed
    128-column-wise on the PE (identity matmul).
  - normalize on device: reciprocal of the denominator row, broadcast
    across the 64 head partitions with a K=1 matmul, multiply.
  - output written h-major ([64, 2048]); host transposes back.
  - emission is interleaved with generators: proj(c) and attn(c-1) alternate
    so the in-order engine queues see attention work during DMA waits, and
    scores run LOOKAHEAD k-tiles ahead of the AV consumer.
"""

import numpy as np

import concourse.bacc as bacc
import concourse.mybir as mybir
import concourse.tile as tile
from concourse import bass2jax

B, S, E, H = 8, 2048, 1024, 64
NCORES = 8
PB = 128  # partition block / k-tile size
QB = 512  # q-chunk (matmul moving free dim)
ET = E // PB  # e-tiles per contraction
QC = S // QB  # q-chunks
KT = S // PB  # k-tiles
DIAG = QB // PB  # diagonal k-tiles per q-chunk

# packed constants layout: columns of the [128, NCONST] "consts" input
C_SB = 0  # [*, 0:2]   scale/bias (rows 0:128)
C_BV = 2  # [*, 2:3]   v bias (rows 0:64)
C_ID = 3  # [*, 3:67]  identity 64x64 (rows 0:64)
C_ONES = 67  # [*, 67:131] ones
C_PERM = 131  # [*, 131:195] row-rebase permutation (p, h) = 1 iff p == h+64
C_MASK = 195  # [*, 195:1091] causal ramp mask: (p, j) = 1 iff j >= p + 384
NCONST = C_MASK + QB + 384

F32 = mybir.dt.float32
F32R = mybir.dt.float32r
AF = mybir.ActivationFunctionType
MUL = mybir.AluOpType.mult

_CACHE: dict = {}

# schedule/buffering knobs (sweepable)
CFG = {
    "lookahead": 4,
    "xbufs": 12,
    "wtbufs": 8,
    "psbufs": 4,
    "dma2": "gpsimd",  # second x-stream queue
    "cs_q": "gpsimd",
    "wqkv_q": "gpsimd",
    "diag_first": False,
    "attn_first": False,
}


def _interleave(*gens):
    """Drive generators round-robin; the first (proj) gets two steps per turn."""
    alive = list(gens)
    steps = {id(g): (2 if i == 0 and len(gens) > 1 else 1) for i, g in enumerate(gens)}
    while alive:
        for g in list(alive):
            for _ in range(steps[id(g)]):
                try:
                    next(g)
                except StopIteration:
                    alive.remove(g)
                    break


def _build_nc():
    nc = bacc.Bacc("TRN2", target_bir_lowering=False, debug=False)
    xT = nc.dram_tensor("xT", [E, S], F32R, kind="ExternalInput").ap()
    wqkv = nc.dram_tensor("wqkv", [E, 3 * H], F32R, kind="ExternalInput").ap()
    consts = nc.dram_tensor("consts", [PB, NCONST], F32R, kind="ExternalInput").ap()
    out = nc.dram_tensor("out", [H + 1, S], F32, kind="ExternalOutput").ap()

    with tile.TileContext(nc) as tc:
        with (
            tc.tile_pool(name="const", bufs=1) as constp,
            tc.tile_pool(name="xs", bufs=CFG["xbufs"]) as xpool,
            tc.tile_pool(name="qkv", bufs=1) as qkvp,
            tc.tile_pool(name="wt", bufs=CFG["wtbufs"]) as wtp,
            tc.tile_pool(name="fin", bufs=2) as finp,
            tc.tile_pool(name="pqk", bufs=1, space="PSUM") as pqk,
            tc.tile_pool(name="pv", bufs=1, space="PSUM") as pvp,
            tc.tile_pool(name="ps", bufs=CFG["psbufs"], space="PSUM") as psp,
            tc.tile_pool(name="pav", bufs=2, space="PSUM") as pavp,
        ):
            # wqkv split in two on the sync queue: the e=0/1 slice unblocks
            # the first projection matmul ~1us earlier than one big transfer.
            wqkv_sb = constp.tile([PB, ET, 3 * H], F32R)
            nc.sync.dma_start(
                wqkv_sb[:, 0:2, :],
                wqkv[0 : 2 * PB, :].rearrange("(t p) m -> p t m", p=PB),
            )
            nc.sync.dma_start(
                wqkv_sb[:, 2:ET, :],
                wqkv[2 * PB :, :].rearrange("(t p) m -> p t m", p=PB),
            )
            cs = constp.tile([PB, NCONST], F32R)

            scale_ap = cs[:, C_SB : C_SB + 1].bitcast(F32)
            bias_ap = cs[:, C_SB + 1 : C_SB + 2].bitcast(F32)
            bv_ap = cs[0:H, C_BV : C_BV + 1].bitcast(F32)
            id_ap = cs[0:H, C_ID : C_ID + H].bitcast(F32)
            ones_ap = cs[:, C_ONES : C_ONES + H]
            perm_ap = cs[:, C_PERM : C_PERM + H]

            qkT = qkvp.tile([PB, S], F32R)  # rows 0:64 = q/8, 64:128 = k
            kT = qkvp.tile([H, S], F32R)  # k re-based to partitions 0:64
            vT = qkvp.tile([H, S], F32)  # v h-major (bias applied)
            vsb = qkvp.tile([PB, KT, H + 1], F32R)  # v k-major + ones col

            def load_consts():
                # emitted after chunk 0's x tiles so the pool queue's first
                # deliveries are the tiles the first accumulation needs
                getattr(nc, CFG["cs_q"]).dma_start(cs[:], consts[:])
                for m in range(KT):
                    nc.vector.tensor_copy(vsb[:, m, H : H + 1], ones_ap[:, 0:1])

            proj_state = {}

            def proj_main(c):
                # DMA + accumulating matmuls only. The PSUM->SBUF epilogue is
                # emitted separately (proj_epi) AFTER the previous chunk's
                # attention has fully emitted: the in-order ACT/DVE queues
                # would otherwise stall ready exps/masks behind the epilogue
                # copies, which wait on the DMA-paced projection matmuls.
                qs = slice(c * QB, (c + 1) * QB)
                p_qk = pqk.tile([PB, QB], F32, tag="pqk")
                p_v = pvp.tile([H, QB], F32, tag="pv")
                proj_state[c] = (p_qk, p_v)
                for e in range(ET):
                    xt = xpool.tile([PB, QB], F32R, tag="xt")
                    dma_eng = nc.sync if e % 2 == 0 else getattr(nc, CFG["dma2"])
                    dma_eng.dma_start(xt[:], xT[e * PB : (e + 1) * PB, qs])
                    nc.tensor.matmul(
                        p_qk[:],
                        wqkv_sb[:, e, 0 : 2 * H],
                        xt[:],
                        start=(e == 0),
                        stop=(e == ET - 1),
                    )
                    nc.tensor.matmul(
                        p_v[:],
                        wqkv_sb[:, e, 2 * H : 3 * H],
                        xt[:],
                        start=(e == 0),
                        stop=(e == ET - 1),
                    )
                    yield

            def proj_epi(c):
                qs = slice(c * QB, (c + 1) * QB)
                p_qk, p_v = proj_state.pop(c)
                # qkT copy on DVE, vT copy on ACT: parallel engines instead of
                # two serialized ACT instructions.
                nc.vector.tensor_scalar(
                    qkT[:, qs],
                    p_qk[:],
                    scale_ap,
                    bias_ap,
                    MUL,
                    mybir.AluOpType.add,
                )
                nc.scalar.activation(vT[:, qs], p_v[:], AF.Identity, bias=bv_ap)
                # re-base k rows 64:128 -> 0:64 via permutation matmul
                p_k = pqk.tile([PB, QB], F32, tag="pqk")
                nc.tensor.matmul(p_k[0:H, :], perm_ap, qkT[:, qs], start=True, stop=True)
                nc.vector.tensor_copy(kT[:, qs], p_k[0:H, :])
                for t in range(DIAG):
                    m = DIAG * c + t
                    p_vt = pvp.tile([PB, H], F32, tag="pv")
                    nc.tensor.transpose(p_vt[:], vT[:, m * PB : (m + 1) * PB], id_ap)
                    nc.vector.tensor_copy(vsb[:, m, 0:H], p_vt[:])

            def attn(c):
                qs = slice(c * QB, (c + 1) * QB)
                nkt = DIAG * c + DIAG
                p_av = pavp.tile([H + 1, QB], F32, tag="pav")

                def weights_tile(m):
                    # scores -> exp -> (diagonal) causal mask
                    p_s = psp.tile([PB, QB], F32, tag="ps")
                    nc.tensor.matmul(
                        p_s[:],
                        kT[:, m * PB : (m + 1) * PB],
                        qkT[0:H, qs],
                        start=True,
                        stop=True,
                    )
                    w = wtp.tile([PB, QB], F32R, tag="w")
                    nc.scalar.activation(w[:], p_s[:], AF.Exp)
                    i = m - DIAG * c
                    if i >= 0:
                        nc.vector.tensor_tensor(
                            w[:],
                            w[:],
                            cs[:, C_MASK + 384 - PB * i : C_MASK + 384 - PB * i + QB],
                            MUL,
                        )
                    return w

                L = CFG["lookahead"]
                if c == QC - 1:
                    # final chunk: diagonals first so the drain of the last
                    # (unpipelined) m-steps has no exp->mask->AV chain
                    order = list(range(DIAG * c, nkt)) + list(range(0, DIAG * c))
                else:
                    order = list(range(nkt))
                ws = {m: weights_tile(m) for m in order[: min(L, nkt)]}
                yield
                for idx, m in enumerate(order):
                    if idx + L < nkt:
                        ws[order[idx + L]] = weights_tile(order[idx + L])
                    nc.tensor.matmul(
                        p_av[:],
                        vsb[:, m, :],
                        ws.pop(m),
                        start=(idx == 0),
                        stop=(idx == nkt - 1),
                    )
                    yield
                # unnormalized output + denominator row; division happens on
                # the host as part of unsharding (out row 64 = denominators)
                osb = finp.tile([H + 1, QB], F32, tag="osb")
                nc.scalar.activation(osb[:], p_av[:], AF.Copy)
                yield
                nc.sync.dma_start(out[:, qs], osb[:])
                yield

            # interleaved emission: proj_main(c) alternates with attn(c-1) so
            # the in-order engine queues see attention work during DMA waits;
            # each projection epilogue is emitted after that attention so no
            # exp/mask queues behind an epilogue copy still waiting on DMA.
            g0 = proj_main(0)
            for _ in range(4):
                next(g0)  # chunk 0's first x tiles lead both DMA queues
            load_consts()
            _interleave(g0)
            proj_epi(0)
            for c in range(1, QC):
                _interleave(proj_main(c), attn(c - 1))
                proj_epi(c)
            _interleave(attn(QC - 1))

    nc.compile()
    return nc


def _host_inputs(x, Wq, bq, Wk, bk, Wv, bv):
    x = np.asarray(x, np.float32)
    Wq, bq = np.asarray(Wq, np.float32), np.asarray(bq, np.float32)
    Wk, bk = np.asarray(Wk, np.float32), np.asarray(bk, np.float32)
    Wv, bv = np.asarray(Wv, np.float32), np.asarray(bv, np.float32)

    sc = np.float32(1.0 / np.sqrt(H))
    wqkv = np.ascontiguousarray(np.concatenate([Wq.T, Wk.T, Wv.T], axis=1))  # [E, 3H]

    cs = np.zeros((PB, NCONST), np.float32)
    cs[:, C_SB] = np.concatenate([np.full(H, sc, np.float32), np.ones(H, np.float32)])
    cs[:, C_SB + 1] = np.concatenate([bq * sc, bk])
    cs[:H, C_BV] = bv
    cs[:H, C_ID : C_ID + H] = np.eye(H, dtype=np.float32)
    cs[:, C_ONES : C_ONES + H] = 1.0
    cs[H:PB, C_PERM : C_PERM + H] = np.eye(H, dtype=np.float32)
    j = np.arange(QB + 384, dtype=np.int64)[None, :]
    k = np.arange(PB, dtype=np.int64)[:, None]
    cs[:, C_MASK:] = (j >= k + 384).astype(np.float32)

    shared = {"wqkv": wqkv, "consts": cs}
    in_maps = []
    for b in range(B):
        m = dict(shared)
        m["xT"] = np.ascontiguousarray(x[b].T)
        in_maps.append(m)
    return in_maps


def get_nc():
    if "nc" not in _CACHE:
        _CACHE["nc"] = _build_nc()
    return _CACHE["nc"]


def kernel(x, Wq, bq, Wk, bk, Wv, bv):
    nc = get_nc()
    in_maps = _host_inputs(x, Wq, bq, Wk, bk, Wv, bv)
    results = bass2jax.run_bass_via_pjrt(nc, in_maps, n_cores=NCORES)
    out = np.empty((B, S, H), np.float32)
    for b in range(B):
        o = results[b]["out"]
        out[b] = (o[:H] / o[H : H + 1]).T
    return out
